# revision 23
# baseline (speedup 1.0000x reference)
"""Trainium2 Bass kernel for nn_Blipv2 (sparse agent attention).

Strategy (data-parallel over batch, one batch element per NeuronCore):

The reference is algebraically restructured so the only O(n*d^2) work is a
single n x 768 x 384 pooling GEMM; everything else collapses to rank-4
matmuls against small per-batch matrices:

  - mean over tokens commutes past the second pooling Linear:
        at = mean_n(gelu(x@W1 + bp1)) @ Wp2^T + bp2,  W1 = Wq^T @ Wp1^T
  - softmax over the head axis (H=1) is identically 1, so the dual-softmax
    "bias fusion" becomes  a * (softmax(s, -1) + 2) / 3.
  - k/v/q projections fold into the 4-row matrices:
        scores_a^T = x @ (SCALE * Wk^T at^T)       (n,4)
        q_scores^T = x @ (SCALE * Wq^T at_mod^T)   (n,4)
        agent_v    = (A_mod @ x) @ Wv^T            (4,768)
        out        = q_attn @ (agent_v @ Wproj^T) + bproj
  All x-products run on the PE in fp16 (exact products, fp32 accumulate).
"""

import sys

for _p in ("/opt/trn_rl_repo", "/root/.axon_site/_ro/trn_rl_repo"):
    if _p not in sys.path:
        sys.path.insert(0, _p)

import numpy as np

import concourse.bacc as bacc
import concourse.bass as bass
import concourse.tile as tile
from concourse import bass_isa, mybir
from concourse.bass_utils import run_bass_kernel_spmd
from concourse.masks import make_identity

F32 = mybir.dt.float32
F16 = mybir.dt.float16
AF = mybir.ActivationFunctionType
ALU = mybir.AluOpType

B = 8          # batch (== n_cores)
N = 4096       # tokens
D = 768        # model dim
A = 4          # agents
PH = 384       # pool hidden
SCALE = D ** -0.5
NT = N // 128  # 32 token tiles
KD = D // 128  # 6 dim tiles
MH = PH // 128  # 3 pool-hidden tiles
OT = (A * D) // 128  # 24 at-output tiles

_CACHED = None


def _bcast_mid(ap2, mid):
    """(P, F) AP -> (P, mid, F) with stride-0 middle dim."""
    return bass.AP(tensor=ap2.tensor, offset=ap2.offset,
                   ap=[ap2.ap[0], [0, mid], ap2.ap[1]])


def _build_program():
    nc = bacc.Bacc("TRN2", target_bir_lowering=False, debug=False, num_devices=B)

    din = {}
    def inp(name, shape, dt):
        din[name] = nc.dram_tensor(name, list(shape), dt, kind="ExternalInput")
        return din[name]

    x16d = inp("x16", (N, D), F16)
    xt16d = inp("xt16", (D, N), F16)
    w1d = inp("w1", (D, PH), F16)          # [d, j]
    wkd = inp("wk", (D, D), F16)           # [d, d'] natural
    wqd = inp("wq", (D, D), F16)
    wcombd = inp("wcomb", (D, D), F16)     # Wv^T @ Wproj^T: [d', d]
    wp2td = inp("wp2t", (PH, A * D), F16)  # Wp2^T: [h, o]
    bp1d = inp("bp1", (PH,), F32)
    bp2d = inp("bp2", (A * D,), F32)
    bpj16d = inp("bpj16", (1, D), F16)
    ones16d = inp("ones16", (1, N), F16)
    a1td = inp("a1t", (N, A), F32)         # attn_1[b,0].T
    a2td = inp("a2t", (N, A), F32)
    t1td = inp("t1t", (D, A), F32)         # agent_tk1[b,0].T
    t2td = inp("t2t", (D, A), F32)

    out_d = nc.dram_tensor("out", [N, D], F16, kind="ExternalOutput")
    aat_d = nc.dram_tensor("aat", [N, A], F32, kind="ExternalOutput")
    atm_d = nc.dram_tensor("atm", [D, A], F32, kind="ExternalOutput")

    from contextlib import ExitStack

    with tile.TileContext(nc) as tc, ExitStack() as stk:
        consts = stk.enter_context(tc.tile_pool(name="consts", bufs=1))
        bigw = stk.enter_context(tc.tile_pool(name="bigw", bufs=1))
        work = stk.enter_context(tc.tile_pool(name="work", bufs=1))

        # ---------------- loads (issue order = consumption order) ----------------
        w1 = bigw.tile([128, KD, PH], F16)
        nc.scalar.dma_start(out=w1[:], in_=w1d.ap().rearrange("(k p) j -> p k j", p=128))
        xT = bigw.tile([128, KD, N], F16)       # x^T, dim on partitions (host-transposed)
        xt_src = xt16d.ap().rearrange("(k p) t -> p k t", p=128)
        for q in range(8):
            nc.sync.dma_start(out=xT[:, :, q * 512:(q + 1) * 512],
                              in_=xt_src[:, :, q * 512:(q + 1) * 512])
        wp2t = bigw.tile([128, MH, A * D], F16)
        nc.sync.dma_start(out=wp2t[:], in_=wp2td.ap().rearrange("(k p) o -> p k o", p=128))
        wk = bigw.tile([128, KD, D], F16)
        nc.sync.dma_start(out=wk[:], in_=wkd.ap().rearrange("(k p) d -> p k d", p=128))
        wq = bigw.tile([128, KD, D], F16)
        nc.sync.dma_start(out=wq[:], in_=wqd.ap().rearrange("(k p) d -> p k d", p=128))
        xN = bigw.tile([128, NT, D], F16)       # x natural, tok on partitions
        nc.sync.dma_start(out=xN[:], in_=x16d.ap().rearrange("(t p) d -> p t d", p=128))
        wcomb = bigw.tile([128, KD, D], F16)
        nc.sync.dma_start(out=wcomb[:], in_=wcombd.ap().rearrange("(k p) d -> p k d", p=128))

        bp1 = consts.tile([128, MH], F32)
        nc.scalar.dma_start(out=bp1[:], in_=bp1d.ap().rearrange("(m p) -> p m", p=128))
        bp2 = consts.tile([128, OT], F32)
        nc.scalar.dma_start(out=bp2[:], in_=bp2d.ap().rearrange("(j p) -> p j", p=128))

        s1 = work.tile([128, NT, A], F32)
        nc.scalar.dma_start(out=s1[:], in_=a1td.ap().rearrange("(t p) a -> p t a", p=128))
        s2 = work.tile([128, NT, A], F32)
        nc.scalar.dma_start(out=s2[:], in_=a2td.ap().rearrange("(t p) a -> p t a", p=128))

        tt1 = work.tile([128, KD, A], F32)
        nc.scalar.dma_start(out=tt1[:], in_=t1td.ap().rearrange("(i p) a -> p i a", p=128))
        tt2 = work.tile([128, KD, A], F32)
        nc.scalar.dma_start(out=tt2[:], in_=t2td.ap().rearrange("(i p) a -> p i a", p=128))
        tT = work.tile([128, KD, A], F32)
        nc.vector.tensor_add(tT[:], tt1[:], tt2[:])

        ident = consts.tile([128, 128], F16)
        make_identity(nc, ident[:])
        ident4 = consts.tile([4, 4], F16)
        make_identity(nc, ident4[:])
        ones_col = consts.tile([128, 1], F32)
        nc.gpsimd.memset(ones_col[:], 1.0)
        ones_row = consts.tile([1, 128], F32)
        nc.vector.memset(ones_row[:], 1.0)

        # ---------------- P1: pooling GEMM + token mean ----------------
        partials = work.tile([128, MH, 8], F32)
        with tc.tile_pool(name="p1ps", bufs=4, space="PSUM") as p1ps, \
             tc.tile_pool(name="p1act", bufs=3) as p1act:
            for nn in range(8):
                for m in range(MH):
                    ps = p1ps.tile([128, 512], F32)
                    for k in range(KD):
                        nc.tensor.matmul(ps[:], w1[:, k, m * 128:(m + 1) * 128],
                                         xT[:, k, nn * 512:(nn + 1) * 512],
                                         start=(k == 0), stop=(k == KD - 1))
                    gact = p1act.tile([128, 512], F32)
                    nc.scalar.activation(gact[:], ps[:], AF.Gelu,
                                         bias=bp1[:, m:m + 1],
                                         accum_out=partials[:, m, nn:nn + 1])
        meanh = work.tile([128, MH], F32)
        nc.vector.tensor_reduce(meanh[:], partials[:], axis=mybir.AxisListType.X,
                                op=ALU.add)
        meanh16 = work.tile([128, MH], F16)
        nc.vector.tensor_scalar_mul(meanh16[:], meanh[:], 1.0 / N)

        # ---------------- P2: at, t-softmax modulation, Mk/Mq ----------------
        atT = work.tile([128, KD, A], F32)   # at^T (pre-modulation), [d, a]
        with tc.tile_pool(name="p2ps", bufs=2, space="PSUM") as p2ps:
            for j in range(OT):
                psa = p2ps.tile([128, 1], F32)
                for kh in range(MH):
                    nc.tensor.matmul(psa[:], wp2t[:, kh, j * 128:(j + 1) * 128],
                                     meanh16[:, kh:kh + 1],
                                     start=(kh == 0), stop=(kh == MH - 1))
                i, a = j % KD, j // KD
                nc.vector.tensor_add(atT[:, i, a:a + 1], psa[:], bp2[:, j:j + 1])

        atTs16 = work.tile([128, KD, A], F16)
        nc.vector.tensor_scalar_mul(atTs16[:], atT[:], SCALE)

        # t-softmax over d (partitions+tiles): exp, cross-partition sum, per-a sum
        expt = work.tile([128, KD, A], F32)
        nc.scalar.activation(expt[:], tT[:], AF.Exp)
        par = work.tile([128, KD, A], F32)
        nc.gpsimd.partition_all_reduce(par[:], expt[:], channels=128,
                                       reduce_op=bass_isa.ReduceOp.add)
        sum_t = work.tile([128, A], F32)
        par_swapped = bass.AP(tensor=par[:].tensor, offset=par[:].offset,
                              ap=[par[:].ap[0], [1, A], [A, KD]])
        nc.vector.tensor_reduce(sum_t[:], par_swapped, axis=mybir.AxisListType.X,
                                op=ALU.add)
        rec_t = work.tile([128, A], F32)
        nc.vector.reciprocal(rec_t[:], sum_t[:])

        smt = work.tile([128, KD, A], F32)
        nc.vector.tensor_tensor(smt[:], expt[:], _bcast_mid(rec_t[:], KD), op=ALU.mult)
        u_at = work.tile([128, KD, A], F32)   # (softmax(t)+2) * at   (missing /3)
        nc.vector.scalar_tensor_tensor(u_at[:], smt[:], 2.0, atT[:],
                                       op0=ALU.add, op1=ALU.mult)
        at_out = work.tile([128, KD, A], F32)
        nc.vector.tensor_scalar_mul(at_out[:], u_at[:], 1.0 / 3.0)
        nc.scalar.dma_start(out=atm_d.ap().rearrange("(i p) a -> p i a", p=128),
                          in_=at_out[:])
        atm16 = work.tile([128, KD, A], F16)  # SCALE * at_mod^T
        nc.vector.tensor_scalar_mul(atm16[:], u_at[:], SCALE / 3.0)

        rhs8 = work.tile([128, KD, 2 * A], F16)   # [Mk^T | Mq^T] per dim tile
        with tc.tile_pool(name="p2b", bufs=2, space="PSUM") as p2b, \
             tc.tile_pool(name="p2c", bufs=2) as p2c:
            mk16 = p2c.tile([A, D], F16, name="mk16")
            mq16 = p2c.tile([A, D], F16, name="mq16")
            for h in range(2):
                psk = p2b.tile([A, 384], F32, name="psk", tag="mkq")
                for k in range(KD):
                    nc.tensor.matmul(psk[:], atTs16[:, k, :],
                                     wk[:, k, h * 384:(h + 1) * 384],
                                     start=(k == 0), stop=(k == KD - 1))
                nc.vector.tensor_copy(mk16[:, h * 384:(h + 1) * 384], psk[:])
                psq = p2b.tile([A, 384], F32, name="psq", tag="mkq")
                for k in range(KD):
                    nc.tensor.matmul(psq[:], atm16[:, k, :],
                                     wq[:, k, h * 384:(h + 1) * 384],
                                     start=(k == 0), stop=(k == KD - 1))
                nc.vector.tensor_copy(mq16[:, h * 384:(h + 1) * 384], psq[:])
            for i in range(KD):
                tpk = p2b.tile([128, A], F16, name="tpk", tag="tpm")
                nc.tensor.transpose(tpk[:], mk16[0:A, i * 128:(i + 1) * 128], ident4[:])
                nc.vector.tensor_copy(rhs8[:, i, 0:A], tpk[:])
                tpq = p2b.tile([128, A], F16, name="tpq", tag="tpm")
                nc.tensor.transpose(tpq[:], mq16[0:A, i * 128:(i + 1) * 128], ident4[:])
                nc.vector.tensor_copy(rhs8[:, i, A:2 * A], tpq[:])

        # ---------------- P3: scores, exps, softmax sums, q_attn ----------------
        st_all = work.tile([128, NT, 12], F32)  # [exp_sa(4) | exp_qs(4) | exp_s(4)]
        s_lane = bass.AP(tensor=st_all[:].tensor, offset=st_all[:, 0, 8:9].offset,
                         ap=[st_all[:].ap[0], [12, NT], [1, A]])
        nc.vector.tensor_add(s_lane, s1[:], s2[:])
        nc.scalar.activation(s_lane, s_lane, AF.Exp)
        qat16 = work.tile([128, NT, A], F16)    # q_attn (normalized), token-major
        qaT = work.tile([5, NT, 128], F16)      # q_attn^T + ones row (for out matmul)
        nc.scalar.dma_start(out=qaT[4:5, :, :], in_=ones16d.ap())  # ones row

        with tc.tile_pool(name="p3ps", bufs=3, space="PSUM") as p3ps, \
             tc.tile_pool(name="p3sum", bufs=1, space="PSUM") as p3sum, \
             tc.tile_pool(name="p3tp", bufs=2, space="PSUM") as p3tp, \
             tc.tile_pool(name="p3v", bufs=3) as p3v:
            sum_ps = p3sum.tile([1, 2 * A], F32)
            for mt in range(NT):
                ps = p3ps.tile([128, 2 * A], F32)
                for k in range(KD):
                    nc.tensor.matmul(ps[:], xT[:, k, mt * 128:(mt + 1) * 128],
                                     rhs8[:, k, :], start=(k == 0), stop=(k == KD - 1))
                nc.scalar.activation(st_all[:, mt, 0:8], ps[:], AF.Exp)
                # accumulate column sums of exp_sa and exp_s across all tokens
                grp = bass.AP(tensor=st_all[:].tensor,
                              offset=st_all[:, mt, 0:1].offset,
                              ap=[st_all[:].ap[0], [8, 2], [1, A]])
                nc.tensor.matmul(sum_ps[:], ones_col[:], grp,
                                 start=(mt == 0), stop=(mt == NT - 1),
                                 skip_group_check=True)
                # q_attn: softmax over agents (free dim)
                qs_sum = p3v.tile([128, 1], F32)
                nc.vector.tensor_reduce(qs_sum[:], st_all[:, mt, 4:8],
                                        axis=mybir.AxisListType.X, op=ALU.add)
                qrec = p3v.tile([128, 1], F32)
                nc.vector.reciprocal(qrec[:], qs_sum[:])
                nc.vector.tensor_scalar_mul(qat16[:, mt, :], st_all[:, mt, 4:8], qrec[:])
                # transpose q_attn tile -> (4, 128) for later use as lhsT
                tp = p3tp.tile([4, 128], F16)
                nc.tensor.transpose(tp[:], qat16[:, mt, :], ident[:])
                nc.vector.tensor_copy(qaT[0:4, mt, :], tp[:])

            sums_sb = work.tile([1, 2 * A], F32)
            nc.vector.tensor_copy(sums_sb[:], sum_ps[:])

        rr1 = work.tile([1, 2 * A], F32)
        nc.vector.reciprocal(rr1[:], sums_sb[:])
        rbc = work.tile([128, 2 * A], F32)
        with tc.tile_pool(name="p3c", bufs=1, space="PSUM") as p3c:
            rbc_ps = p3c.tile([128, 2 * A], F32)
            nc.tensor.matmul(rbc_ps[:], ones_row[:], rr1[:], start=True, stop=True)
            nc.vector.tensor_copy(rbc[:], rbc_ps[:])

        bc1 = work.tile([128, A], F32)   # rec_a * rec_s / 3
        nc.vector.scalar_tensor_tensor(bc1[:], rbc[:, 0:A], 1.0 / 3.0, rbc[:, A:2 * A],
                                       op0=ALU.mult, op1=ALU.mult)
        bc2 = work.tile([128, A], F32)   # rec_a * 2 / 3
        nc.vector.tensor_scalar_mul(bc2[:], rbc[:, 0:A], 2.0 / 3.0)

        aa_all = work.tile([128, NT, A], F32)   # agent_rep^T
        amod16 = work.tile([128, NT, A], F16)   # A_mod^T
        sa_lane = bass.AP(tensor=st_all[:].tensor, offset=st_all[:].offset,
                          ap=[st_all[:].ap[0], [12, NT], [1, A]])
        def bc_nt(ap2):
            return bass.AP(tensor=ap2.tensor, offset=ap2.offset,
                           ap=[ap2.ap[0], [0, NT], ap2.ap[1]])
        with tc.tile_pool(name="p3b", bufs=1) as p3b:
            nc.vector.tensor_tensor(aa_all[:], sa_lane, bc_nt(rbc[:, 0:A]), op=ALU.mult)
            tmp_all = p3b.tile([128, NT, A], F32)
            nc.vector.tensor_tensor(tmp_all[:], sa_lane, s_lane, op=ALU.mult)
            nc.vector.tensor_tensor(tmp_all[:], tmp_all[:], bc_nt(bc1[:]), op=ALU.mult)
            tmp2_all = p3b.tile([128, NT, A], F32)
            nc.vector.tensor_tensor(tmp2_all[:], sa_lane, bc_nt(bc2[:]), op=ALU.mult)
            nc.vector.tensor_add(amod16[:], tmp_all[:], tmp2_all[:])
        nc.scalar.dma_start(out=aat_d.ap().rearrange("(t p) a -> p t a", p=128),
                          in_=aa_all[:])

        # ---------------- P4: agent_xv -> agent_v -> avp ----------------
        def transpose_4xD(src16, dst, pool_ps):
            # src16: (4, D) f16 SBUF -> dst (128, KD, A) f16
            for i in range(KD):
                tp = pool_ps.tile([128, A], F16)
                nc.tensor.transpose(tp[:], src16[0:A, i * 128:(i + 1) * 128], ident4[:])
                nc.vector.tensor_copy(dst[:, i, :], tp[:])

        avp = work.tile([5, D], F16)
        nc.scalar.dma_start(out=avp[4:5, :], in_=bpj16d.ap())  # bias row
        with tc.tile_pool(name="p4ps", bufs=2, space="PSUM") as p4ps, \
             tc.tile_pool(name="p4sb", bufs=2) as p4sb:
            # agent_xv = A_mod @ x  (contract over tokens)
            axv_ps = [p4ps.tile([A, 384], F32, name=f"axv{_h}", tag="axv") for _h in range(2)]
            for k in range(NT):
                for h in range(2):
                    nc.tensor.matmul(axv_ps[h][:], amod16[:, k, :],
                                     xN[:, k, h * 384:(h + 1) * 384],
                                     start=(k == 0), stop=(k == NT - 1))
            axv16 = p4sb.tile([A, D], F16)
            for h in range(2):
                nc.vector.tensor_copy(axv16[:, h * 384:(h + 1) * 384], axv_ps[h][:])
            axvT = p4sb.tile([128, KD, A], F16)
            transpose_4xD(axv16, axvT, p4ps)

            # avp = agent_xv @ (Wv^T Wproj^T)   [Wcomb folded on host]
            avp_ps = [p4ps.tile([A, 384], F32, name=f"avp{_h}", tag="avp") for _h in range(2)]
            for k in range(KD):
                for h in range(2):
                    nc.tensor.matmul(avp_ps[h][:], axvT[:, k, :],
                                     wcomb[:, k, h * 384:(h + 1) * 384],
                                     start=(k == 0), stop=(k == KD - 1))
            for h in range(2):
                nc.vector.tensor_copy(avp[0:A, h * 384:(h + 1) * 384], avp_ps[h][:])

        # ---------------- P5: out = q_attn_ext @ avp_ext ----------------
        out_view = out_d.ap().rearrange("(t p) d -> p t d", p=128)
        with tc.tile_pool(name="p5ps", bufs=4, space="PSUM") as p5ps, \
             tc.tile_pool(name="p5sb", bufs=3) as p5sb:
            for mt in range(NT):
                ob = p5sb.tile([128, D], F16)
                for h in range(2):
                    ps = p5ps.tile([128, 384], F32)
                    nc.tensor.matmul(ps[:], qaT[:, mt, :], avp[:, h * 384:(h + 1) * 384],
                                     start=True, stop=True)
                    eng = nc.vector if h == 0 else nc.scalar
                    if h == 0:
                        nc.vector.tensor_copy(ob[:, h * 384:(h + 1) * 384], ps[:])
                    else:
                        nc.scalar.copy(ob[:, h * 384:(h + 1) * 384], ps[:])
                nc.sync.dma_start(out=out_view[:, mt, :], in_=ob[:])

    nc.compile()
    return nc


def _host_prep(inputs):
    x = np.asarray(inputs["x"], np.float32)
    Wqkv = np.asarray(inputs["Wqkv"], np.float32)
    Wq, Wk, Wv = Wqkv[0:D], Wqkv[D:2 * D], Wqkv[2 * D:3 * D]
    Wp1 = np.asarray(inputs["Wp1"], np.float32)
    Wp2 = np.asarray(inputs["Wp2"], np.float32)
    Wproj = np.asarray(inputs["Wproj"], np.float32)

    w1 = (Wq.astype(np.float64).T @ Wp1.astype(np.float64).T).astype(np.float16)
    shared = {
        "w1": np.ascontiguousarray(w1),
        "wk": np.ascontiguousarray(Wk.astype(np.float16)),
        "wq": np.ascontiguousarray(Wq.astype(np.float16)),
        "wcomb": np.ascontiguousarray(
            (Wv.astype(np.float64).T @ Wproj.astype(np.float64).T).astype(np.float16)),
        "wp2t": np.ascontiguousarray(Wp2.T.astype(np.float16)),
        "bp1": np.ascontiguousarray(np.asarray(inputs["bp1"], np.float32)),
        "bp2": np.ascontiguousarray(np.asarray(inputs["bp2"], np.float32)),
        "ones16": np.ones((1, N), np.float16),
        "bpj16": np.ascontiguousarray(np.asarray(inputs["bproj"], np.float32).astype(np.float16).reshape(1, D)),
    }
    a1 = np.asarray(inputs["attn_1"], np.float32)
    a2 = np.asarray(inputs["attn_2"], np.float32)
    t1 = np.asarray(inputs["agent_tk1"], np.float32)
    t2 = np.asarray(inputs["agent_tk2"], np.float32)

    in_maps = []
    for b in range(B):
        m = dict(shared)
        m["x16"] = np.ascontiguousarray(x[b].astype(np.float16))
        m["xt16"] = np.ascontiguousarray(m["x16"].T)
        m["a1t"] = np.ascontiguousarray(a1[b, 0].T)
        m["a2t"] = np.ascontiguousarray(a2[b, 0].T)
        m["t1t"] = np.ascontiguousarray(t1[b, 0].T)
        m["t2t"] = np.ascontiguousarray(t2[b, 0].T)
        in_maps.append(m)
    return in_maps


def kernel(**inputs):
    global _CACHED
    if _CACHED is None:
        _CACHED = _build_program()
    nc = _CACHED
    in_maps = _host_prep(inputs)
    res = run_bass_kernel_spmd(nc, in_maps, list(range(B)))

    out = np.empty((B, N, D), np.float32)
    agent_rep = np.empty((B, 1, A, N), np.float32)
    at = np.empty((B, 1, A, D), np.float32)
    for b in range(B):
        r = res.results[b]
        out[b] = r["out"].astype(np.float32)
        agent_rep[b, 0] = r["aat"].T
        at[b, 0] = r["atm"].T
    return out, agent_rep, at
